# revision 19
# baseline (speedup 1.0000x reference)
"""GRU (nn_RNN) Trainium2 Bass kernel — data-parallel over batch on 8 NeuronCores.

Per core: B_local=8 of the 64-row batch, full T=512 recurrence.
  Phase 1: igates = x @ w_ih.T + b      (big f32r matmul, stored bf16 in DRAM)
  Phase 2: GRU recurrence               (col-tiled f32r matmuls, PE bias injects)
  Phase 3: ys = all_outs @ w_lin.T + b  (big f32r matmul)

Layouts (per core):
  xT      [I, BL*T]   x transposed host-side (bt b-major)
  whhT_g  [H, 3H]     w_hh.T with gate cols regrouped into 8 blocks of
                      [hr_c | hz_c | hn_c] (c = H-chunk of 128)
  igates  [T, 8, BL, 384] bf16 DRAM scratch, same 384-blocks
  h state grouped on chip: partition 32j+b holds H-chunks c=j and c=4+j
  hT      [128, (c,b)] transposed h, lhsT for the recurrent matmuls
  aoT     [8, 128, BL, T] all_outs.T DRAM scratch (phase 3 lhsT tiles)
"""

import numpy as np
import ml_dtypes

import concourse.bass as bass
import concourse.mybir as mybir
import concourse.tile as tile
from concourse import bacc
from concourse.bass_utils import run_bass_kernel_spmd

f32 = mybir.dt.float32
f32r = mybir.dt.float32r
bf16 = mybir.dt.bfloat16
AF = mybir.ActivationFunctionType

B, T_FULL, I_DIM, H, O = 64, 512, 512, 1024, 512
NCORES = 8
BL = B // NCORES          # 8
G = 3 * H                 # 3072
KT = H // 128             # 8 k-tiles





def build(T, do_p1=True, do_p2=True, do_p3=True, do_tr=True, do_inj=True, nj=4, ntr=4, dbg=False):
    BT = BL * T
    MT = BT // 128          # bt-tiles (one b each when T >= 128)
    QT = T // 128           # 128-row t-chunks per b
    assert T % 128 == 0
    FL = 64                 # hist flush period
    NFL = T // FL

    nc = bacc.Bacc("TRN2", target_bir_lowering=False, debug=False,
                   num_devices=NCORES)

    xT = nc.dram_tensor("xT", [I_DIM, BT], f32r, kind="ExternalInput").ap()
    wihT = nc.dram_tensor("wihT", [I_DIM, G], f32r, kind="ExternalInput").ap()
    whhT = nc.dram_tensor("whhT", [H, G], bf16, kind="ExternalInput").ap()
    wlinT = nc.dram_tensor("wlinT", [H, O], f32r, kind="ExternalInput").ap()
    brow = nc.dram_tensor("brow", [1, G], f32r, kind="ExternalInput").ap()
    bng = nc.dram_tensor("bng", [1, H], bf16, kind="ExternalInput").ap()
    blin = nc.dram_tensor("blin", [1, O], f32r, kind="ExternalInput").ap()
    id8 = nc.dram_tensor("id8", [8, 8], bf16, kind="ExternalInput").ap()
    on8 = nc.dram_tensor("on8", [1, 8], bf16, kind="ExternalInput").ap()
    idf = nc.dram_tensor("idf", [128, 128], f32, kind="ExternalInput").ap()
    on128 = nc.dram_tensor("on128", [1, 128], f32r, kind="ExternalInput").ap()
    z256 = nc.dram_tensor("z256", [128, 256], bf16, kind="ExternalInput").ap()
    z1 = nc.dram_tensor("z1", [1, 128], bf16, kind="ExternalInput").ap()

    hout = nc.dram_tensor("hout", [BL, H], f32, kind="ExternalOutput").ap()
    ys = nc.dram_tensor("ys", [BT, O], f32, kind="ExternalOutput").ap()
    if dbg:
        dbg_ig = nc.dram_tensor("dbg_ig", [T * 8 * BL * 384], bf16,
                                kind="ExternalOutput").ap()
        dbg_ao = nc.dram_tensor("dbg_ao", [KT * 128 * BL * T], f32r,
                                kind="ExternalOutput").ap()
        dbg_ps = nc.dram_tensor("dbg_ps", [2, 128, 512], f32,
                                kind="ExternalOutput").ap()
        dbg_psn = nc.dram_tensor("dbg_psn", [2, 128, 128], f32,
                                 kind="ExternalOutput").ap()
        dbg_ig2 = nc.dram_tensor("dbg_ig2", [8, G], bf16,
                                 kind="ExternalOutput").ap()

    with tile.TileContext(nc) as tc:
        with (
            tc.tile_pool(name="dram", bufs=1, space="DRAM") as dpool,
            tc.tile_pool(name="consts", bufs=1) as cpool,
            tc.tile_pool(name="whh", bufs=1) as whpool,
        ):
            igates = dpool.tile([T, 8, BL, 384], bf16, tag="igates")
            aoT = dpool.tile([KT, 128, BL, T], f32r, tag="aoT")

            # constants / weights resident in SBUF
            id8_sb = cpool.tile([8, 8], bf16, tag="id8")
            nc.sync.dma_start(id8_sb[:], id8)
            on8_sb = cpool.tile([1, 8], bf16, tag="on8")
            nc.sync.dma_start(on8_sb[:], on8)
            idf_sb = cpool.tile([128, 128], f32, tag="idf")
            nc.sync.dma_start(idf_sb[:], idf)
            bng_sb = cpool.tile([1, H], bf16, tag="bng")
            nc.sync.dma_start(bng_sb[:], bng)
            ones1 = cpool.tile([1, 128], f32r, tag="ones1")
            nc.sync.dma_start(ones1[:], on128)
            z1_sb = cpool.tile([1, 128], bf16, tag="z1")
            nc.sync.dma_start(z1_sb[:], z1)

            whh_sb = []
            for k in range(KT):
                w = whpool.tile([128, G], bf16, tag=f"whh{k}")
                nc.sync.dma_start(w[:], whhT[128 * k:128 * (k + 1), :])
                whh_sb.append(w)

            # ---------------- Phase 1: igates ----------------
            with (
                tc.tile_pool(name="p1w", bufs=1) as p1w,
                tc.tile_pool(name="p1x", bufs=2) as p1x,
                tc.tile_pool(name="p1s", bufs=3) as p1s,
                tc.tile_pool(name="p1ps", bufs=3, space="PSUM") as p1ps,
            ):
                wih_sb = []
                for k in range(4):
                    w = p1w.tile([128, G], f32r, tag=f"wih{k}")
                    nc.sync.dma_start(w[:], wihT[128 * k:128 * (k + 1), :])
                    wih_sb.append(w)
                brow_sb = p1w.tile([1, G], f32r, tag="brow")
                nc.sync.dma_start(brow_sb[:], brow)

                for m in range(MT if do_p1 else 0):
                    b_idx, q = divmod(m, QT)
                    xts = []
                    for k in range(4):
                        xt = p1x.tile([128, 128], f32r, tag=f"xt{k}")
                        nc.sync.dma_start(
                            xt[:], xT[128 * k:128 * (k + 1),
                                      128 * m:128 * (m + 1)])
                        xts.append(xt)
                    stg = p1s.tile([128, G], bf16, tag="stg")
                    for n in range(6):
                        ps = p1ps.tile([128, 512], f32, tag="ps1")
                        for k in range(4):
                            nc.tensor.matmul(
                                ps[:], xts[k][:],
                                wih_sb[k][:, 512 * n:512 * (n + 1)],
                                start=(k == 0), stop=False,
                                skip_group_check=True)
                        nc.tensor.matmul(
                            ps[:], ones1[:],
                            brow_sb[:, 512 * n:512 * (n + 1)],
                            start=False, stop=True, skip_group_check=True)
                        nc.vector.tensor_copy(
                            stg[:, 512 * n:512 * (n + 1)], ps[:])
                    dst = igates[q * 128:(q + 1) * 128, :, b_idx, :]
                    nc.sync.dma_start(
                        dst, stg.rearrange("p (c f) -> p c f", c=8))

            # ---------------- Phase 2: recurrence ----------------
            with (
                tc.tile_pool(name="state", bufs=1) as stpool,
                tc.tile_pool(name="hT", bufs=2) as hpool,
                tc.tile_pool(name="hist", bufs=2) as histpool,
                tc.tile_pool(name="ig", bufs=3) as igpool,
                tc.tile_pool(name="ps", bufs=3, space="PSUM") as pspool,
                tc.tile_pool(name="psn", bufs=4, space="PSUM") as psnpool,
                tc.tile_pool(name="psT", bufs=1, space="PSUM") as pstpool,
                tc.tile_pool(name="ew", bufs=2) as ewpool,
            ):
                h_sb = stpool.tile([128, 256], f32, tag="h")
                nc.vector.memset(h_sb[:], 0.0)
                hT = hpool.tile([128, 256], bf16, tag="hT")
                nc.sync.dma_start(hT[:], z256)

                hist = None
                for t in range(T if do_p2 else 0):
                    tu = t % FL
                    if tu == 0:
                        hist = histpool.tile([128, KT * BL * FL], f32r,
                                             tag="hist")

                    ig = igpool.tile([8, G], bf16, tag="ig")
                    nc.sync.dma_start(
                        ig.rearrange("b (c f) -> b c f", c=8),
                        igates[t].rearrange("c b f -> b c f"))

                    if do_tr:
                        psT = pstpool.tile([128, 256], f32, tag="psT")
                        hT_new = hpool.tile([128, 256], bf16, tag="hT")
                    else:
                        hT_new = hT

                    for r in range(2):
                        ps = pspool.tile([128, 512], f32, tag="ps")
                        psn = psnpool.tile([128, 128], f32, tag="psn")
                        nc.tensor.matmul(
                            ps[:, 0:384], z1_sb[:, :],
                            bng_sb[0:1, 0:384], start=True, stop=False,
                            skip_group_check=True)
                        nc.tensor.matmul(
                            psn[:, :], z1_sb[:, :],
                            bng_sb[0:1, 0:128], start=True, stop=False,
                            skip_group_check=True)
                        for k in range(KT):
                            for j in range(nj):
                                c = r * 4 + j
                                nc.tensor.matmul(
                                    ps[32 * j:32 * (j + 1), 0:384],
                                    hT[:, 32 * k:32 * (k + 1)],
                                    whh_sb[k][:, 384 * c:384 * (c + 1)],
                                    start=False, stop=False,
                                    tile_position=(0, 32 * j),
                                    skip_group_check=True)
                        for j in range(nj if do_inj else 0):
                            c = r * 4 + j
                            nc.tensor.matmul(
                                ps[32 * j:32 * j + 8, 0:256],
                                id8_sb[:, :],
                                ig[:, 384 * c:384 * c + 256],
                                start=False, stop=False,
                                tile_position=(0, 32 * j),
                                skip_group_check=True)
                            nc.tensor.matmul(
                                ps[32 * j:32 * j + 8, 256:384],
                                on8_sb[:, :],
                                bng_sb[:, 128 * c:128 * (c + 1)],
                                start=False, stop=(j == 3),
                                tile_position=(0, 32 * j),
                                skip_group_check=True)
                            nc.tensor.matmul(
                                psn[32 * j:32 * j + 8, :],
                                id8_sb[:, :],
                                ig[:, 384 * c + 256:384 * (c + 1)],
                                start=False, stop=(j == 3),
                                tile_position=(0, 32 * j),
                                skip_group_check=True)

                        if dbg and t == 0:
                            dstage = ewpool.tile([128, 512], f32,
                                                 tag=f"dstage{r}")
                            nc.vector.tensor_copy(dstage[:], ps[:])
                            nc.sync.dma_start(dbg_ps[r], dstage[:])
                            dstage2 = ewpool.tile([128, 128], f32,
                                                  tag=f"dstage2{r}")
                            nc.vector.tensor_copy(dstage2[:], psn[:])
                            nc.sync.dma_start(dbg_psn[r], dstage2[:])
                            if r == 0:
                                nc.sync.dma_start(dbg_ig2, ig[:])
                        # gate math: h = n*(1-z) + z*h with
                        #   1-z computed as sigmoid(-pre_z)
                        rz = ewpool.tile([128, 256], f32, tag="rz")
                        nc.scalar.activation(rz[:], ps[:, 0:256], AF.Sigmoid)
                        zc = ewpool.tile([128, 128], f32, tag="zc")
                        nc.scalar.activation(zc[:], ps[:, 128:256],
                                             AF.Sigmoid, scale=-1.0)
                        tt = ewpool.tile([128, 128], f32, tag="tt")
                        nc.vector.tensor_mul(tt[:], rz[:, 0:128],
                                             ps[:, 256:384])
                        pn = ewpool.tile([128, 128], f32, tag="pn")
                        nc.vector.tensor_add(pn[:], tt[:], psn[:])
                        nt = ewpool.tile([128, 128], f32, tag="nt")
                        nc.scalar.activation(nt[:], pn[:], AF.Sigmoid,
                                             scale=2.0)
                        t1 = ewpool.tile([128, 128], f32, tag="t1")
                        nc.vector.tensor_mul(
                            t1[:], rz[:, 128:256],
                            h_sb[:, 128 * r:128 * (r + 1)])
                        # h = zc*(2*s-1) + z*h = (2*(zc*s) - zc) + z*h
                        vv = ewpool.tile([128, 128], f32, tag="vv")
                        nc.vector.tensor_mul(vv[:], zc[:], nt[:])
                        w2 = ewpool.tile([128, 128], f32, tag="w2")
                        nc.vector.scalar_tensor_tensor(
                            w2[:], vv[:], 2.0, zc[:],
                            mybir.AluOpType.mult, mybir.AluOpType.subtract)
                        nc.vector.tensor_add(
                            h_sb[:, 128 * r:128 * (r + 1)], w2[:], t1[:])

                    if do_tr:
                        for r in range(2):
                            nc.tensor.transpose(
                                psT[:, 128 * r:128 * (r + 1)],
                                h_sb[:, 128 * r:128 * (r + 1)],
                                idf_sb[:, :])
                            nc.vector.tensor_copy(
                                hT_new[:, 128 * r:128 * (r + 1)],
                                psT[:, 128 * r:128 * (r + 1)])
                        src_t = psT.rearrange("p (r j v) -> p r j v",
                                              r=2, j=4)[:, :, :, 0:BL]
                        nc.vector.tensor_copy(
                            hist.rearrange("p (r j b u) -> p r j b u",
                                           r=2, j=4, b=BL)[:, :, :, :, tu],
                            src_t)
                    if tu == FL - 1 and do_tr:
                        fi = t // FL
                        for k in range(KT):
                            nc.sync.dma_start(
                                aoT[k, :, :, fi * FL:(fi + 1) * FL],
                                hist.rearrange("p (k b u) -> p k b u",
                                               k=KT, b=BL)[:, k])
                    hT = hT_new

                # final hidden state out
                for j in range(4):
                    nc.sync.dma_start(
                        hout.rearrange("b (r j2 col) -> j2 b r col",
                                       r=2, j2=4)[j],
                        h_sb[32 * j:32 * j + 8].rearrange(
                            "b (r col) -> b r col", r=2))

            if dbg:
                nc.sync.dma_start(dbg_ig, igates.flatten())
                nc.sync.dma_start(dbg_ao, aoT.flatten())

            # ---------------- Phase 3: output projection ----------------
            with (
                tc.tile_pool(name="p3w", bufs=1) as p3w,
                tc.tile_pool(name="p3x", bufs=2) as p3x,
                tc.tile_pool(name="p3s", bufs=3) as p3s,
                tc.tile_pool(name="p3ps", bufs=3, space="PSUM") as p3ps,
            ):
                wlin_sb = []
                for k in range(KT):
                    w = p3w.tile([128, O], f32r, tag=f"wlin{k}")
                    nc.sync.dma_start(w[:], wlinT[128 * k:128 * (k + 1), :])
                    wlin_sb.append(w)
                blin_sb = p3w.tile([1, O], f32r, tag="blin")
                nc.sync.dma_start(blin_sb[:], blin)

                for m in range(MT if do_p3 else 0):
                    b_idx, q = divmod(m, QT)
                    lts = []
                    for k in range(KT):
                        lt = p3x.tile([128, 128], f32r, tag=f"lt{k}")
                        nc.sync.dma_start(
                            lt[:], aoT[k, :, b_idx, q * 128:(q + 1) * 128])
                        lts.append(lt)
                    ps = p3ps.tile([128, 512], f32, tag="ps3")
                    for k in range(KT):
                        nc.tensor.matmul(ps[:], lts[k][:],
                                         wlin_sb[k][:],
                                         start=(k == 0), stop=False,
                                         skip_group_check=True)
                    nc.tensor.matmul(ps[:], ones1[:], blin_sb[:],
                                     start=False, stop=True,
                                     skip_group_check=True)
                    ystg = p3s.tile([128, O], f32, tag="ystg")
                    nc.vector.tensor_copy(ystg[:], ps[:])
                    nc.sync.dma_start(ys[m * 128:(m + 1) * 128, :], ystg[:])

    nc.compile()
    return nc


_CACHE = {}


def get_program(T=T_FULL):
    if T not in _CACHE:
        _CACHE[T] = build(T)
    return _CACHE[T]


def _cgroup(w):
    """[X, 3H] gate-major cols -> c-grouped 384-blocks [hr_c|hz_c|hn_c]."""
    X = w.shape[0]
    return np.ascontiguousarray(
        w.reshape(X, 3, 8, 128).transpose(0, 2, 1, 3).reshape(X, G))


def make_in_maps(input, w_ih, w_hh, b, b_n, w_lin, b_lin, T=T_FULL):
    x = np.asarray(input, np.float32)
    wihT_g = _cgroup(np.ascontiguousarray(np.asarray(w_ih, np.float32).T))
    whhT_g = _cgroup(np.ascontiguousarray(
        np.asarray(w_hh, np.float32).T)).astype(ml_dtypes.bfloat16)
    brow_g = _cgroup(np.asarray(b, np.float32).reshape(1, G))
    wlinT = np.ascontiguousarray(np.asarray(w_lin, np.float32).T)
    bng = np.asarray(b_n, np.float32).reshape(1, H).astype(ml_dtypes.bfloat16)
    blin_row = np.asarray(b_lin, np.float32).reshape(1, O)
    id8 = np.eye(8, dtype=ml_dtypes.bfloat16)
    on8 = np.ones((1, 8), dtype=ml_dtypes.bfloat16)
    idf = np.eye(128, dtype=np.float32)

    in_maps = []
    for core in range(NCORES):
        sl = slice(core * BL, (core + 1) * BL)
        xc = np.ascontiguousarray(
            x[sl, :T].transpose(2, 0, 1).reshape(I_DIM, BL * T))
        in_maps.append({
            "xT": xc, "wihT": wihT_g, "whhT": whhT_g, "wlinT": wlinT,
            "brow": brow_g, "bng": bng, "blin": blin_row,
            "id8": id8, "on8": on8, "idf": idf,
            "on128": np.ones((1, 128), np.float32),
            "z256": np.zeros((128, 256), ml_dtypes.bfloat16),
            "z1": np.zeros((1, 128), ml_dtypes.bfloat16),
        })
    return in_maps


def assemble(results, T=T_FULL):
    out_h = np.concatenate([r["hout"] for r in results], axis=0)
    out_ys = np.concatenate(
        [r["ys"].reshape(BL, T, O) for r in results], axis=0)
    return out_h, out_ys


def kernel(input, w_ih, w_hh, b, b_n, w_lin, b_lin):
    nc = get_program(T_FULL)
    in_maps = make_in_maps(input, w_ih, w_hh, b, b_n, w_lin, b_lin, T_FULL)
    res = run_bass_kernel_spmd(nc, in_maps, core_ids=list(range(NCORES)))
    return assemble(res.results, T_FULL)
